# revision 2
# baseline (speedup 1.0000x reference)
"""Trainium2 Bass kernel for the 3-group sparse attention module.

Shapes: x [4, 1024, 768], H=8 heads, head_dim 96 split into 3 groups of 32.
  qkv = x @ W_qkv -> q,k,v [B,H,N,96]; groups q3..q5/k3..k5/v3..v5 (32 each)
  x3 = attend(q4, [k3,k4], [v3,v4]); x4 = attend(q5, [k3,k5], [v3,v5])
  x5 = attend(q5, [k4,k5], [v4,v5]);  out = [x3|x4|x5] @ W_proj + b_proj
  scale = 96 ** -0.5

Sharding: 8 cores = 4 batches x 2 query-halves (no collectives), identical
SPMD graph per core (host permutes each core's rows first; see _prep_inputs).

Architecture (v2): the kernel is ScalarE-bound -- 21M probabilities must go
through the ACT exp LUT at 1 elem/lane/cycle.  Everything is organised to
keep ACT 100% busy on maximally-wide activations:
  * scores S^T[m,nq] stream into a 6-bank PSUM ring ([128,1536] f32 x2);
    one exp per 3 banks (N=1536 amortises the ~352-cycle ACT overhead).
  * score matmuls are 4-way row-tiled (K=32, tile_position=(kpo,0)).
  * AV matmuls are 2-way column-tiled and packed into TWO psum banks per
    head: bankY = y3[0:33] | y4[64:97], bankZ = T[0:33] | y5[64:97].
    Banks are DVE-memset to zero and every AV matmul uses start=False --
    per-element has_written semantics make first-touch an overwrite of the
    zeros, avoiding the bank-wide clear a start=True would do.
  * the exp-consuming AV stream trails the exp producer by a few groups;
    E tiles ([128,1536] bf16) buffer ~1 head of lag.
  * qkv generation runs as low-priority PE filler on the AV banks' psum
    slots; q/k for head 0 are produced eagerly so ACT starts within ~6us.
  * warmup matmuls during the input DMA keep the PE HAM clock warm.
"""

import numpy as np
import ml_dtypes

B, N, C, H = 4, 1024, 768, 8
HD = 96          # head dim
G = 32           # group dim
NQ = 512         # query rows per core
SCALE = float(HD) ** -0.5
P = 128
NCORES = 8
CH = C // P      # 6 chunks of 128 along contraction/channel dims

_CACHE = {}

# tunables
RING_G = 3       # psum banks (512-col score slices) per exp instruction
E_BUFS = 18      # E-tile rotation depth (buffers AV lag behind exp)
AV_EVERY = 2     # drain the AV backlog every this many exp groups
AV_TRAIL = 3     # slices of pipeline lag kept between exp and AV
WARMUP_MM = 12   # HAM-warming matmuls issued during the input DMA


def _build_graph():
    import concourse.bass as bass
    import concourse.tile as tile
    from concourse import bacc, mybir

    f32 = mybir.dt.float32
    bf16 = mybir.dt.bfloat16

    nc = bacc.Bacc()

    xt_d = nc.declare_dram_parameter("xt", [C, N], bf16, isOutput=False)
    wq_d = nc.declare_dram_parameter("wq", [C, 768], bf16, isOutput=False)
    wk_d = nc.declare_dram_parameter("wk", [C, 768], bf16, isOutput=False)
    wv_d = nc.declare_dram_parameter("wv", [C, 768], bf16, isOutput=False)
    wp_d = nc.declare_dram_parameter("wp", [C, C], bf16, isOutput=False)
    bias_d = nc.declare_dram_parameter("bias", [P, C], f32, isOutput=False)
    out_d = nc.declare_dram_parameter("out", [NQ, C], f32, isOutput=True)

    with tile.TileContext(nc) as tc:
        with (
            tc.tile_pool(name="wgt", bufs=1) as wgt,
            tc.tile_pool(name="acts", bufs=1) as acts,
            tc.tile_pool(name="epool", bufs=E_BUFS) as epool,
            tc.tile_pool(name="small", bufs=2) as small,
            tc.tile_pool(name="outp", bufs=2) as outp,
            tc.tile_pool(name="ring", bufs=2, space="PSUM") as ring,
            tc.tile_pool(name="avps", bufs=1, space="PSUM") as avps,
        ):
            # ---- stage inputs in SBUF (xt/wq/wk first: head-0 prereqs) ----
            xt = [wgt.tile([P, N], bf16, name=f"xt{i}") for i in range(CH)]
            wq = [wgt.tile([P, 768], bf16, name=f"wq{i}") for i in range(CH)]
            wk = [wgt.tile([P, 768], bf16, name=f"wk{i}") for i in range(CH)]
            wv = [wgt.tile([P, 768], bf16, name=f"wv{i}") for i in range(CH)]
            wp = [wgt.tile([P, C], bf16, name=f"wp{i}") for i in range(CH)]
            bias = wgt.tile([P, C], f32, name="bias")
            for i in range(CH):
                nc.sync.dma_start(xt[i][:], xt_d[P * i:P * (i + 1), :])
                nc.sync.dma_start(wq[i][:], wq_d[P * i:P * (i + 1), :])
                nc.sync.dma_start(wk[i][:], wk_d[P * i:P * (i + 1), :])
            for i in range(CH):
                nc.sync.dma_start(wv[i][:], wv_d[P * i:P * (i + 1), :])
                nc.sync.dma_start(wp[i][:], wp_d[P * i:P * (i + 1), :])
            nc.sync.dma_start(bias[:], bias_d[:])

            # ---- persistent activation tensors ----
            # qT: [768, 512]  per head h: [q4; q5; q5]
            q_sb = [acts.tile([P, NQ], bf16, name=f"q{i}") for i in range(CH)]
            # kT: [768, 1024] per head: [k3; k5; k4]
            k_sb = [acts.tile([P, N], bf16, name=f"k{i}") for i in range(CH)]
            # qT copy #2 per head: band0 <- q5, band2 <- q4
            q2_sb = [acts.tile([P, NQ], bf16, name=f"q2_{i}") for i in range(CH)]
            # v natural per m-tile: 24 groups of [v_g | 1] (33 cols each)
            v_sb = [acts.tile([P, 24 * 33], bf16, name=f"v{i}") for i in range(8)]
            # unnormalized y^T (bf16) channels: 256*g + 32h + d
            u_sb = [acts.tile([P, NQ], bf16, name=f"u{i}") for i in range(CH)]

            def band(h, j):
                p = 96 * h + 32 * j
                return p // P, p % P

            # ---- PE warmup during the input DMA (keeps HAM at 8/8) ----
            warm = wgt.tile([P, NQ], bf16, name="warm")
            nc.vector.memset(warm[:], 0.25)
            for i in range(WARMUP_MM):
                wps = ring.tile([P, NQ], f32, tag="ring", name="warmps")
                nc.tensor.matmul(wps[:], lhsT=warm[:, 0:P], rhs=warm[:],
                                 start=True, stop=True)

            # ---- generation helpers ----
            def gen_q(co, pool, tag):
                ps = pool.tile([P, NQ], f32, tag=tag, name="qps")
                for ci in range(CH):
                    nc.tensor.matmul(
                        ps[:], lhsT=wq[ci][:, P * co:P * (co + 1)],
                        rhs=xt[ci][:, 0:NQ],
                        start=(ci == 0), stop=(ci == CH - 1))
                nc.vector.tensor_copy(q_sb[co][:], ps[:])

            def gen_k(co, pool, tags):
                for nh in range(2):
                    ps = pool.tile([P, NQ], f32, tag=tags[nh], name="kps")
                    for ci in range(CH):
                        nc.tensor.matmul(
                            ps[:], lhsT=wk[ci][:, P * co:P * (co + 1)],
                            rhs=xt[ci][:, NQ * nh:NQ * (nh + 1)],
                            start=(ci == 0), stop=(ci == CH - 1))
                    nc.vector.tensor_copy(k_sb[co][:, NQ * nh:NQ * (nh + 1)],
                                          ps[:])

            def gen_q2(h):
                # band2 <- q4 (q_sb band 0), band0 <- q5 (q_sb band 1)
                for dst_j, src_j in ((2, 0), (0, 1)):
                    dti, dpo = band(h, dst_j)
                    sti, spo = band(h, src_j)
                    nc.vector.tensor_copy(
                        q2_sb[dti][dpo:dpo + G, :], q_sb[sti][spo:spo + G, :])

            def gen_v(mt):
                # two psum pieces: cols 0-511 (groups 0-15), 512-767 (16-23)
                vdst = v_sb[mt][:].rearrange("p (g d) -> p g d", d=33)
                for half, w, g0, tag in ((0, NQ, 0, "Y"), (NQ, 256, 16, "Z")):
                    ps = avps.tile([P, w], f32, tag=tag, name="vps")
                    for ci in range(CH):
                        nc.tensor.matmul(
                            ps[:], lhsT=xt[ci][:, P * mt:P * (mt + 1)],
                            rhs=wv[ci][:, half:half + w],
                            start=(ci == 0), stop=(ci == CH - 1))
                    ng = w // G
                    nc.vector.tensor_copy(
                        vdst[:, g0:g0 + ng, 0:32],
                        ps[:].rearrange("p (g d) -> p g d", d=32))
                nc.vector.memset(vdst[:, :, 32:33], 1.0)

            # prologue: head-0 prerequisites, eagerly (ring psums)
            gen_q(0, ring, "ring")
            gen_k(0, ring, ("ring", "ring"))
            gen_q2(0)

            # everything else: low-priority PE filler on the AV psum banks
            with tc.high_priority(offset=-1000000):
                gen_k(1, avps, ("Y", "Z"))
                gen_q(1, avps, "Y")
                gen_q2(1)
                for mt in range(8):
                    gen_v(mt)
                for co in range(2, CH):
                    gen_q(co, avps, "Y")
                    gen_k(co, avps, ("Z", "Y"))
                    for h2 in range(H):
                        if (96 * (h2 + 1) - 1) // P == co:
                            gen_q2(h2)

            # ---- attention: ACT-paced ring of scores -> exp -> AV ----
            # score blocks per head, ordered for row-band rotation:
            #   a=(k3,q4) d=(k5,q5) e=(k4,q5) c=(k3,q5*) b=(k4,q4*)
            BLOCKS = [
                ("a", 0, 0, 0),   # name, kj, qj, qsrc_idx (0=q_sb, 1=q2_sb)
                ("d", 1, 1, 0),
                ("e", 2, 2, 0),
                ("c", 0, 0, 1),
                ("b", 2, 2, 1),
            ]
            # AV target region per block: (bank_idx, part_off, vg_off)
            #   bank 0 = Y (y3 | y4), bank 1 = Z (T | y5)
            AV_REGION = {"a": (0, 0, 0), "b": (0, 0, 1), "c": (0, 64, 0),
                         "d": (1, 0, 2), "e": (1, 64, 1)}
            # MMs per region per head (for stop flags)
            REGION_TOTAL = {(0, 0): 16, (0, 64): 8, (1, 0): 8, (1, 64): 8}

            qsrcs = (q_sb, q2_sb)
            head_banks = {}        # h -> (bankY, bankZ)
            region_count = {}      # (h, bank_idx, off) -> emitted count
            head_left = {h: 40 for h in range(H)}
            ready = []             # (h, name, mt, et, sli) exp'd score slices
            drained = 0
            state = {"ps": None, "fill": 0, "pending": [], "groups": 0}

            def emit_normalize(h):
                bankY, bankZ = head_banks[h]
                t_sb = small.tile([33, NQ], f32, tag="tsb")
                nc.vector.tensor_copy(t_sb[:], bankZ[0:33, :])
                y4c = small.tile([33, NQ], f32, tag="y4c")
                nc.vector.tensor_copy(y4c[:], bankY[64:97, :])
                y5c = small.tile([33, NQ], f32, tag="y5c")
                nc.vector.tensor_copy(y5c[:], bankZ[64:97, :])
                ysum4 = small.tile([33, NQ], f32, tag="ysum4")
                nc.vector.tensor_add(ysum4[:], y4c[:], t_sb[:])
                ysum5 = small.tile([33, NQ], f32, tag="ysum5")
                nc.vector.tensor_add(ysum5[:], y5c[:], t_sb[:])

                zb = small.tile([96, NQ], f32, tag="zb")
                nc.vector.tensor_copy(zb[0:1, :], bankY[32:33, :])
                nc.vector.tensor_copy(zb[32:33, :], ysum4[32:33, :])
                nc.vector.tensor_copy(zb[64:65, :], ysum5[32:33, :])
                rz = small.tile([96, NQ], f32, tag="rz")
                nc.vector.reciprocal(rz[:], zb[:])

                for g, ysrc in ((0, bankY), (1, ysum4), (2, ysum5)):
                    rzb = small.tile([G, NQ], f32, tag="rzb")
                    if g == 0:
                        rzsrc = rz
                    else:
                        rzsrc = small.tile([1, NQ], f32, tag="rzsrc")
                        nc.vector.tensor_copy(rzsrc[:], rz[32 * g:32 * g + 1, :])
                    nc.gpsimd.partition_broadcast(rzb[:], rzsrc[0:1, :])
                    ch = 256 * g + 32 * h
                    nc.vector.tensor_mul(
                        u_sb[ch // P][ch % P:ch % P + G, :],
                        ysrc[0:32, :], rzb[:])

            def emit_av(h, name, mt, et, sli):
                if h not in head_banks:
                    bankY = avps.tile([P, NQ], f32, tag="Y", name=f"bY{h}")
                    bankZ = avps.tile([P, NQ], f32, tag="Z", name=f"bZ{h}")
                    nc.vector.memset(bankY[:], 0.0)
                    nc.vector.memset(bankZ[:], 0.0)
                    head_banks[h] = (bankY, bankZ)
                bank_idx, po, vg = AV_REGION[name]
                bank = head_banks[h][bank_idx]
                key = (h, bank_idx, po)
                cnt = region_count.get(key, 0)
                region_count[key] = cnt + 1
                last = cnt + 1 == REGION_TOTAL[(bank_idx, po)]
                gg = 3 * h + vg
                nc.tensor.matmul(
                    bank[po:po + 33, :],
                    lhsT=v_sb[mt][:, 33 * gg:33 * gg + 33],
                    rhs=et[:, NQ * sli:NQ * (sli + 1)],
                    start=False, stop=last, skip_group_check=True,
                    tile_position=(0, po))
                head_left[h] -= 1
                if head_left[h] == 0:
                    emit_normalize(h)

            def drain_av(upto):
                nonlocal drained
                while drained < upto:
                    emit_av(*ready[drained])
                    drained += 1

            def emit_exp():
                ps = state["ps"]
                nf = state["fill"] * NQ
                et = epool.tile([P, RING_G * NQ], bf16, tag="e", name="et")
                nc.scalar.activation(
                    et[:, 0:nf], ps[:, 0:nf],
                    mybir.ActivationFunctionType.Exp, scale=SCALE)
                for (h, nm, mt, sli) in state["pending"]:
                    ready.append((h, nm, mt, et, sli))
                state["ps"] = None
                state["fill"] = 0
                state["pending"] = []
                state["groups"] += 1
                if state["groups"] % AV_EVERY == 0:
                    drain_av(max(0, len(ready) - AV_TRAIL))

            def emit_score(h, name, kj, qj, qsi, mt):
                if state["ps"] is None:
                    state["ps"] = ring.tile([P, RING_G * NQ], f32, tag="ring",
                                            name="sps")
                sl = state["ps"][:, NQ * state["fill"]:NQ * (state["fill"] + 1)]
                kti, kpo = band(h, kj)
                qti, qpo = band(h, qj)
                nc.tensor.matmul(
                    sl, lhsT=k_sb[kti][kpo:kpo + G, P * mt:P * (mt + 1)],
                    rhs=qsrcs[qsi][qti][qpo:qpo + G, :],
                    start=True, stop=True,
                    tile_position=(kpo, 0))
                state["pending"].append((h, name, mt, state["fill"]))
                state["fill"] += 1
                if state["fill"] == RING_G:
                    emit_exp()

            for h in range(H):
                for mt in range(8):
                    for (name, kj, qj, qsi) in BLOCKS:
                        emit_score(h, name, kj, qj, qsi, mt)
            if state["fill"]:
                emit_exp()
            drain_av(len(ready))

            # ---- projection + bias ----
            for nt in range(4):
                ps = ring.tile([P, C], f32, tag="ring", name="pps")
                for half, w in ((0, 512), (512, 256)):
                    for ci in range(CH):
                        nc.tensor.matmul(
                            ps[:, half:half + w],
                            lhsT=u_sb[ci][:, P * nt:P * (nt + 1)],
                            rhs=wp[ci][:, half:half + w],
                            start=(ci == 0), stop=(ci == CH - 1))
                o_sb = outp.tile([P, C], f32, tag="osb")
                nc.vector.tensor_add(o_sb[:], ps[:], bias[:])
                nc.sync.dma_start(out_d[P * nt:P * (nt + 1), :], o_sb[:])

    nc.finalize()
    return nc


def _prep_inputs(x, W_qkv, W_proj, b_proj):
    bf16 = ml_dtypes.bfloat16
    # wq: per head [q4, q5, q5] (96 cols); wk: per head [k3, k5, k4]
    qcols, kcols = [], []
    for h in range(H):
        qb, kb = HD * h, C + HD * h
        qcols += list(range(qb + 32, qb + 64)) + 2 * list(range(qb + 64, qb + 96))
        kcols += (list(range(kb, kb + 32)) + list(range(kb + 64, kb + 96))
                  + list(range(kb + 32, kb + 64)))
    wq = np.ascontiguousarray(W_qkv[:, qcols]).astype(bf16)
    wk = np.ascontiguousarray(W_qkv[:, kcols]).astype(bf16)
    wv = np.ascontiguousarray(W_qkv[:, 2 * C:3 * C]).astype(bf16)
    wp = np.ascontiguousarray(W_proj).astype(bf16)
    bias = np.broadcast_to(np.asarray(b_proj, np.float32), (P, C)).copy()

    in_maps = []
    for core in range(NCORES):
        b, half = core // 2, core % 2
        xb = np.asarray(x[b], np.float32)
        xp = np.concatenate([xb[NQ * half:NQ * (half + 1)],
                             xb[NQ * (1 - half):NQ * (2 - half)]], axis=0)
        xt = np.ascontiguousarray(xp.T).astype(bf16)
        in_maps.append({"xt": xt, "wq": wq, "wk": wk, "wv": wv, "wp": wp,
                        "bias": bias})
    return in_maps


def kernel(x, W_qkv, W_proj, b_proj, t_h=None, t_w=None, s_h=None, s_w=None,
           **_unused):
    from concourse.bass_utils import run_bass_kernel_spmd

    if "nc" not in _CACHE:
        _CACHE["nc"] = _build_graph()
    nc = _CACHE["nc"]

    in_maps = _prep_inputs(np.asarray(x), np.asarray(W_qkv),
                           np.asarray(W_proj), np.asarray(b_proj))
    res = run_bass_kernel_spmd(nc, in_maps, core_ids=list(range(NCORES)))
    _CACHE["last_results"] = res

    out = np.empty((B, N, C), np.float32)
    for core in range(NCORES):
        b, half = core // 2, core % 2
        out[b, NQ * half:NQ * (half + 1), :] = res.results[core]["out"]
    return out
